# revision 53
# baseline (speedup 1.0000x reference)
"""Trainium2 Bass kernel for nn_CotLayer (CoT attention layer, dense_cnn).

Sharding: 8-way frame split.  Core (2*clip + half) owns 4 temporal
frames of one clip (half 0: frames 0-3, half 1: frames 4-7).  All ops
except the split-attention GAP are frame-local given a 1-frame halo on
x, so the only cross-core traffic is a [C,1] AllReduce between core
pairs [[0,1],[2,3],[4,5],[6,7]] right before the SE attention, which
both pair members then compute redundantly.

The temporal grouped conv and the dynamic-aggregation unfold read x /
v at frame offsets n-1..n+1; each core's x shard carries 6 frame slots
(own 4 + 1 halo each side, zero-padded at clip edges).  Uniform code
across cores: pad slots hold x=0 and a zeroed BN-bias column, so their
tap contributions vanish without per-core branches.

Everything stays SBUF-resident between passes (k2d, agg, wd) -- no
DRAM spills.  Inputs are packed into 2 device arguments (xall f16 =
x-shard tile-major + all f16 weights; cb32 = f32 constants) since
per-argument dispatch overhead is a measurable share of call time.
"""
import sys
import numpy as np

try:
    import concourse.bass as bass  # noqa: F401
except ImportError:
    sys.path.insert(0, "/opt/trn_rl_repo")

import concourse.bass as bass
import concourse.tile as tile
from concourse import mybir, bacc
from concourse.bass_utils import run_bass_kernel_spmd

# ---- problem constants (hardcoded per spec) ----
C = 128          # channels
NB = 8           # temporal frames per clip
B = 4            # clips
H = W = 64
KS = 3
G = 32           # groupnorm groups = C//4
KC = 96          # KS * C//4 dynamic-kernel channels
EPS = 1e-5
L = 4            # local frames per core
S = 6            # x shard slots (L + halo, with zero pads at clip edges)
PXF = H * W      # pixels per frame = 4096
PT = 512         # pixel tile
NT = PXF // PT   # 8 tiles per frame
NI = NT * L      # 32 (tile, frame) iterations per core
NCORES = 8
GROUPS = [[0, 1], [2, 3], [4, 5], [6, 7]]

F32 = mybir.dt.float32
F16 = mybir.dt.float16
AF = mybir.ActivationFunctionType
ALU = mybir.AluOpType
AXL = mybir.AxisListType

_CACHE = {}

# ---- xall layout: x block [NT, S, PT] tile-major, then f16 weights,
# then the "f32" constants stored as f16 (converted back on device; the
# 2e-2 tolerance leaves plenty of room for f16-rounded BN constants) ----
B16 = NT * S * PT          # 24576 cols of x data
L16 = {
    "wkey": (C, B16 + 0, KS * C),      # (i, tap*o) flattened taps
    "we1a": (C, B16 + 384, C // 2),
    "we1b": (C, B16 + 448, C // 2),
    "we2": (C, B16 + 512, KC),         # w_e2^T stacked twice (rows 0:64/64:128)
    "wv": (C, B16 + 608, C),
    "bkg": (KC, B16 + 736, KS * C),    # (r, tap*c) flattened taps
}
NC16 = 1120
# offsets into the on-device f32 constants tile (filled by one f16->f32
# converting copy from the tail of xall; epsv is a device-side memset)
L32 = {
    "tk": (C, 0, 1),
    "tvc": (C, 1, S),          # per-core: BN bias per slot, 0 at pad slots
    "gavg": (KC, 7, G),
    "c1": (G, 39, C),
    "c96": (G, 167, KC),
    "be2": (KC, 263, 1),
    "ecols": (C, 264, KS * 3),
    "s2": (C, 273, 1),
    "t2": (C, 274, 1),
    "wse1": (C, 275, C),
    "b1": (C, 403, 1),
    "wsed": (C, 404, C),
    "bd": (C, 532, 1),
    "te2": (C, 533, 1),
    "tmask": (C, 534, KS * L),   # per-core 0/1: tap (k,n) valid on this core
}
X32 = 546
XT = B16 + NC16 + X32


def _build_program(single=False, use_cc=True, reps=1):
    cc = use_cc and not single
    nc = bacc.Bacc("TRN2", target_bir_lowering=False, debug=False,
                   num_devices=1 if single else NCORES)

    x_d = nc.dram_tensor("xall", [C, XT], F16, kind="ExternalInput").ap()
    out_d = nc.dram_tensor("out", [C, reps, L, PXF], F16,
                           kind="ExternalOutput").ap()

    with tile.TileContext(nc) as tc:
        with tc.tile_pool(name="consts", bufs=1) as cp, \
             tc.tile_pool(name="statp", bufs=2) as stp, \
             tc.tile_pool(name="xw", bufs=NT) as xw, \
             tc.tile_pool(name="dramp", bufs=2, space="DRAM") as dram:

            cb16 = cp.tile([C, XT - B16], F16, tag="cb16")
            nc.sync.dma_start(cb16[:], x_d[:, B16:XT])
            cb32 = cp.tile([C, X32], F32, tag="cb32")
            nc.vector.tensor_copy(cb32[:], cb16[:, NC16:NC16 + X32])
            epsv = cp.tile([G, 1], F32, tag="epsv")
            nc.gpsimd.memset(epsv[:], EPS)

            def v16(name):
                r, c0, cn = L16[name]
                return cb16[0:r, c0 - B16:c0 - B16 + cn]

            def v32(name):
                r, c0, cn = L32[name]
                return cb32[0:r, c0:c0 + cn]

            def tap16(name, k):
                r, c0, cn = L16[name]
                w = cn // KS
                c0 -= B16
                return cb16[0:r, c0 + k * w:c0 + (k + 1) * w]

            # row KC of wd_big is constant 1.0: the s_all matmuls then add
            # the GN bias tb for free, keeping every PSUM evacuation a plain
            # pair-wide op (gpsimd cannot touch PSUM, so evac width matters)
            wd_big = cp.tile([KC + 1, NI, PT], F16, tag="wd_big",
                             name="wd_big")
            ones_t = cp.tile([1, PT], F16, tag="ones")
            nc.gpsimd.memset(ones_t[:], 1.0)
            k2_all = cp.tile([C, L, PXF], F16, tag="k2_all", name="k2_all")
            ag_all = cp.tile([C, L, PXF], F16, tag="ag_all", name="ag_all")

            for _rep in range(reps):
              stats_buf = stp.tile([KC, NT, L, 6], F32, tag="stats",
                                   name="stats_buf")
              # 16 pair cols from pass A's k2 relus + 8 tile cols from
              # pass B's silus; the reduce sums every column, so the tile
              # must hold exactly the written set
              gap_cols = stp.tile([C, NT * 2 + NT], F32, tag="gapc",
                                  name="gap_cols")
              s_all = [stp.tile([KC + 1, L, C], F16, tag=f"sall{k}",
                                name=f"sall{k}")
                       for k in range(KS)]

              # ================= PASS A =================
              # local frame n sits at shard slot n+1; taps use slots n..n+2
              xs_tiles = []
              with tc.tile_pool(name="ewA", bufs=3) as ew, \
                   tc.tile_pool(name="psK", bufs=2, space="PSUM") as psK, \
                   tc.tile_pool(name="psW", bufs=1, space="PSUM") as psW, \
                   tc.tile_pool(name="psE", bufs=2, space="PSUM") as psA:
                  for t in range(NT):
                      xs_t = xw.tile([C, S, PT], F16, tag="xA", name="xs_t")
                      nc.sync.dma_start(
                          xs_t[:], x_d[:, t * S * PT:(t + 1) * S * PT])
                      xs_tiles.append(xs_t)
                      tc0, tc1 = t * PT, (t + 1) * PT
                      # frames in pairs: 2-bank PSUM tiles make every
                      # evacuation a single [C, 2*PT] instruction
                      for np_ in range(L // 2):
                          n0 = 2 * np_
                          idxp = t * 2 + np_
                          # --- key embed: temporal grouped conv ---
                          ps_k2 = psK.tile([C, 2, PT], F32, tag="ps_k")
                          for h in range(2):
                              for k in range(KS):
                                  nc.tensor.matmul(
                                      ps_k2[:, h, :], tap16("wkey", k),
                                      xs_t[:, n0 + h + k, :],
                                      start=(k == 0), stop=(k == KS - 1))
                          nc.scalar.activation(
                              k2_all[:, n0:n0 + 2, tc0:tc1], ps_k2[:],
                              AF.Relu, bias=v32("tk"),
                              accum_out=gap_cols[:, idxp:idxp + 1])
                          # --- e = relu(bn(w_e1 @ [x; k2d])) ---
                          ps_e = psA.tile([C, PT], F32, tag="ps_e")
                          et = ew.tile([C, PT], F16, tag="eA")
                          for h in range(2):
                              hr = ps_e[h * 64:(h + 1) * 64, :]
                              nc.tensor.matmul(hr, v16("we1a"),
                                               xs_t[:, n0 + h + 1, :],
                                               start=True, stop=False)
                              nc.tensor.matmul(hr, v16("we1b"),
                                               k2_all[:, n0 + h, tc0:tc1],
                                               start=False, stop=True)
                          # relu(ps_e + te2), alternating DVE/Act
                          if (t + np_) % 2 == 0:
                              nc.vector.tensor_scalar(
                                  out=et[:], in0=ps_e[:], scalar1=v32("te2"),
                                  scalar2=0.0, op0=ALU.add, op1=ALU.max)
                          else:
                              nc.scalar.activation(et[:], ps_e[:], AF.Relu,
                                                   bias=v32("te2"))
                          # --- wd = w_e2 @ e (raw; b_e2 via stats) ---
                          ps_w2 = psW.tile([KC, 2, PT], F32, tag="ps_w")
                          r0, c0, cn = L16["we2"]
                          c0 -= B16
                          for h in range(2):
                              nc.tensor.matmul(
                                  ps_w2[:, h, :],
                                  cb16[h * 64:(h + 1) * 64, c0:c0 + cn],
                                  et[h * 64:(h + 1) * 64, :],
                                  start=True, stop=True)
                          idx0 = t * L + n0
                          nc.scalar.activation(
                              wd_big[0:KC, idx0:idx0 + 2, :], ps_w2[:],
                              AF.Copy)
                          for h in range(2):
                              nc.vector.bn_stats(
                                  stats_buf[:, t, n0 + h, :],
                                  wd_big[0:KC, idx0 + h, :])

              # constant row of wd_big, filled while pass A drains (only
              # pass B reads it; issued here to stay off pass A's x loads)
              if _rep == 0:
                  nc.sync.dma_start(
                      wd_big[KC:KC + 1, :, :],
                      ones_t[:].unsqueeze(1).broadcast_to((1, NI, PT)))

              # v production is stats-independent: open its pools around
              # the stats section and prefetch tile 0's v into the bubble
              with tc.tile_pool(name="vw", bufs=2) as vw, \
                   tc.tile_pool(name="psV", bufs=1, space="PSUM") as psV:
                tvc_c = L32["tvc"][1]

                def makev2(t, vtile, j0):
                    # v for slots j0, j0+1 of tile t (bias is slot-uniform;
                    # clip-edge taps are zeroed via tmask in s_all/tb)
                    ps_v = psV.tile([C, 2, PT], F32, tag="ps_v")
                    for h in range(2):
                        nc.tensor.matmul(ps_v[:, h, :], v16("wv"),
                                         xs_tiles[t][:, j0 + h, :],
                                         start=True, stop=True)
                    if (t + j0 // 2) % 2 == 0:
                        nc.vector.tensor_scalar(
                            out=vtile[:, j0:j0 + 2, :], in0=ps_v[:],
                            scalar1=cb32[:, tvc_c:tvc_c + 1],
                            scalar2=None, op0=ALU.add)
                    else:
                        nc.scalar.activation(
                            vtile[:, j0:j0 + 2, :], ps_v[:], AF.Identity,
                            bias=cb32[:, tvc_c:tvc_c + 1])

                vts = {}
                for tt in (0, 1):
                    vts[tt] = vw.tile([C, S, PT], F16, tag="v",
                                      name=f"vpre{tt}")
                    for j0 in (0, 2, 4):
                        makev2(tt, vts[tt], j0)

                # ============ GroupNorm stats (core-local) ============
                with tc.tile_pool(name="stw", bufs=1) as sw, \
                     tc.tile_pool(name="psS", bufs=1, space="PSUM") as psS:
                  mv = sw.tile([KC, L, 2], F32, tag="mv")
                  for n in range(L):
                      nc.vector.bn_aggr(mv[:, n, :], stats_buf[:, :, n, :])
                  # per-channel true mean (+b_e2) and E[x^2]
                  mm96 = sw.tile([KC, 2 * L], F32, tag="mm96")
                  nc.vector.tensor_scalar(
                      out=mm96[:, 0:L], in0=mv[:, :, 0], scalar1=v32("be2"),
                      scalar2=None, op0=ALU.add)
                  sq = sw.tile([KC, L], F32, tag="sq")
                  nc.vector.tensor_mul(sq[:], mm96[:, 0:L], mm96[:, 0:L])
                  nc.vector.tensor_add(mm96[:, L:], mv[:, :, 1], sq[:])
                  # group stats via avg matmul
                  ps_g = psS.tile([G, 2 * L], F32, tag="ps_g")
                  nc.tensor.matmul(ps_g[:], v32("gavg"), mm96[:],
                                   start=True, stop=True)
                  mv32 = sw.tile([G, 2 * L], F32, tag="mv32")
                  nc.scalar.activation(mv32[:, 0:L], ps_g[:, 0:L], AF.Copy)
                  var = sw.tile([G, L], F32, tag="var")
                  nc.vector.tensor_mul(var[:], mv32[:, 0:L], mv32[:, 0:L])
                  nc.vector.tensor_sub(var[:], ps_g[:, L:], var[:])
                  nc.scalar.activation(var[:], var[:], AF.Sqrt,
                                       bias=epsv[:])
                  nc.vector.reciprocal(mv32[:, L:], var[:])
                  # broadcast: rs to 96 rows; mu/rs to 128 channels
                  ps96 = psS.tile([KC, L], F32, tag="ps96")
                  nc.tensor.matmul(ps96[:], v32("c96"), mv32[:, L:],
                                   start=True, stop=True)
                  rs96 = sw.tile([KC, L], F32, tag="rs96")
                  nc.scalar.activation(rs96[:], ps96[:], AF.Copy)
                  psc1 = psS.tile([C, 2 * L], F32, tag="psc1")
                  nc.tensor.matmul(psc1[:], v32("c1"), mv32[:],
                                   start=True, stop=True)
                  mbrb = sw.tile([C, 2 * L], F32, tag="mbrb")
                  nc.scalar.activation(mbrb[:], psc1[:], AF.Copy)
                  # t-bias in channel layout then scatter into S rows
                  tb = sw.tile([C, KS, L], F16, tag="tb")
                  tba = sw.tile([C, L], F32, tag="tba")
                  tbb = sw.tile([C, L], F32, tag="tbb")
                  for k in range(KS):
                      ec = L32["ecols"][1]
                      nc.vector.tensor_scalar(
                          out=tba[:], in0=mbrb[:, 0:L], scalar1=-1.0,
                          scalar2=cb32[:, ec + k * 3:ec + k * 3 + 1],
                          op0=ALU.mult, op1=ALU.add)
                      nc.vector.tensor_mul(tbb[:], tba[:], mbrb[:, L:])
                      nc.vector.tensor_scalar(
                          out=tbb[:], in0=tbb[:],
                          scalar1=cb32[:, ec + k * 3 + 1:ec + k * 3 + 2],
                          scalar2=None, op0=ALU.mult)
                      nc.vector.tensor_scalar(
                          out=tb[:, k, :], in0=tbb[:],
                          scalar1=cb32[:, ec + k * 3 + 2:ec + k * 3 + 3],
                          scalar2=None, op0=ALU.add)
                      # zero the bias of clip-edge taps (pad v slot != 0
                      # now that the v bias is slot-uniform)
                      mc = L32["tmask"][1] + k * L
                      nc.vector.tensor_mul(tb[:, k, :], tb[:, k, :],
                                           cb32[:, mc:mc + L])
                  # frame-major order: frame 0's stationaries land first so
                  # pass B's first tap matmuls unblock as early as possible
                  for n in range(L):
                      for k in range(KS):
                          mc = L32["tmask"][1] + k * L + n
                          eng = nc.vector if (k + n) % 2 == 0 else nc.gpsimd
                          eng.tensor_scalar(
                              out=s_all[k][0:KC, n, :], in0=tap16("bkg", k),
                              scalar1=rs96[:, n:n + 1],
                              scalar2=cb32[0:KC, mc:mc + 1],
                              op0=ALU.mult, op1=ALU.mult)
                          nc.sync.dma_start(s_all[k][KC:KC + 1, n, :],
                                            tb[:, k, n:n + 1])

                # ================= PASS B =================
                with tc.tile_pool(name="ew2", bufs=2) as ew2, \
                     tc.tile_pool(name="mw", bufs=2) as mw, \
                     tc.tile_pool(name="psB", bufs=1, space="PSUM") as psB:
                  for t in range(NT):
                      # v slots live in one tile so tap reads can span a
                      # frame pair as a single [C, 2, PT] operand
                      if t in vts:
                          vtile = vts[t]
                      else:
                          vtile = vw.tile([C, S, PT], F16, tag="v")
                          makev2(t, vtile, 0)
                      a3q = mw.tile([C, L, PT], F16, tag="accB1")
                      for np_ in range(L // 2):
                          n0 = 2 * np_
                          idx0 = t * L + n0
                          if t not in vts:
                              makev2(t, vtile, n0 + 2)
                          terms = []
                          for k in range(KS):
                              ps_w = psB.tile([C, 2, PT], F32,
                                              tag=f"ps_w{k}")
                              for h in range(2):
                                  nc.tensor.matmul(
                                      ps_w[:, h, :],
                                      s_all[k][:, n0 + h, :],
                                      wd_big[:, idx0 + h, :],
                                      start=True, stop=True)
                              m = mw.tile([C, 2, PT], F16, tag=f"m{k}")
                              vsl = vtile[:, n0 + k:n0 + k + 2, :]
                              if k < 2:
                                  nc.vector.tensor_mul(m[:], ps_w[:], vsl)
                              else:
                                  wdf = ew2.tile([C, 2, PT], F16,
                                                 tag="wdf")
                                  nc.scalar.activation(wdf[:], ps_w[:],
                                                       AF.Copy)
                                  nc.gpsimd.tensor_mul(m[:], wdf[:], vsl)
                              terms.append(m)
                          a2 = mw.tile([C, 2, PT], F16, tag="accB0")
                          nc.gpsimd.tensor_add(a2[:], terms[0][:],
                                               terms[1][:])
                          nc.gpsimd.tensor_add(a3q[:, n0:n0 + 2, :],
                                               a2[:], terms[2][:])
                      # one quad-wide silu per tile; gap column per tile
                      idxq = NT * 2 + t
                      nc.scalar.activation(
                          ag_all[:, :, t * PT:(t + 1) * PT],
                          a3q[:], AF.Silu, bias=v32("t2"),
                          scale=v32("s2"),
                          accum_out=gap_cols[:, idxq:idxq + 1])

              # ============ GAP all-reduce + SE attention ============
              with tc.tile_pool(name="sew", bufs=1) as se, \
                   tc.tile_pool(name="ow", bufs=4) as ow, \
                   tc.tile_pool(name="psE2", bufs=1, space="PSUM") as psE:
                  gap = se.tile([C, 1], F32, tag="gap")
                  nc.vector.tensor_reduce(gap[:], gap_cols[:], AXL.XYZW,
                                          ALU.add)
                  gap2 = se.tile([C, 1], F32, tag="gap2")
                  if cc:
                      bin_ = dram.tile([C, 1], F32)
                      bout = dram.tile([C, 1], F32)
                      nc.sync.dma_start(bin_[:], gap[:])
                      nc.gpsimd.collective_compute(
                          "AllReduce", ALU.add, replica_groups=GROUPS,
                          ins=[bin_.opt()], outs=[bout.opt()])
                      nc.sync.dma_start(gap2[:], bout[:])
                  else:
                      nc.gpsimd.tensor_copy(gap2[:], gap[:])

                  # pass C part 1: agg -= k2, in place (independent of the
                  # collective -> overlaps its latency)
                  CH = 2 * PT
                  for ci in range(PXF // CH):
                      sl = ag_all[:, :, ci * CH:(ci + 1) * CH]
                      eng = nc.gpsimd if ci % 2 == 0 else nc.vector
                      eng.tensor_sub(sl, sl,
                                     k2_all[:, :, ci * CH:(ci + 1) * CH])

                  ps_a = psE.tile([C, 1], F32, tag="ps_a")
                  nc.tensor.matmul(ps_a[:], v32("wse1"), gap2[:],
                                   start=True, stop=True)
                  at = se.tile([C, 1], F32, tag="at")
                  nc.scalar.activation(at[:], ps_a[:], AF.Relu,
                                       bias=v32("b1"),
                                       scale=1.0 / (NB * PXF))
                  ps_d = psE.tile([C, 1], F32, tag="ps_d")
                  nc.tensor.matmul(ps_d[:], v32("wsed"), at[:],
                                   start=True, stop=True)
                  sa = se.tile([C, 1], F32, tag="sa")
                  nc.scalar.activation(sa[:], ps_d[:], AF.Sigmoid,
                                       bias=v32("bd"))

                  # pass C part 2: out = sa*(agg-k2) + k2
                  # (3-input STT only runs on DVE; Pool takes a 2-op split)
                  for ci in range(PXF // CH):
                      o = ow.tile([C, L, CH], F16, tag="ots")
                      sl = ag_all[:, :, ci * CH:(ci + 1) * CH]
                      k2sl = k2_all[:, :, ci * CH:(ci + 1) * CH]
                      if ci % 2 == 0:
                          nc.vector.scalar_tensor_tensor(
                              out=o[:], in0=sl, scalar=sa[:], in1=k2sl,
                              op0=ALU.mult, op1=ALU.add)
                      else:
                          nc.gpsimd.tensor_scalar(
                              out=o[:], in0=sl, scalar1=sa[:],
                              scalar2=None, op0=ALU.mult)
                          nc.gpsimd.tensor_add(o[:], o[:], k2sl)
                      nc.sync.dma_start(
                          out_d[:, _rep, :, ci * CH:(ci + 1) * CH], o[:])

    nc.compile()
    return nc


def _host_constants(inp):
    f = np.float32
    d = {}
    s_k = (inp["bnk_g"] / np.sqrt(inp["bnk_v"] + EPS)).astype(f)
    t_k = (inp["bnk_b"] - inp["bnk_m"] * s_k).astype(f)
    w_key = inp["w_key"].reshape(C, C // 4, KS)          # (o, i_local, tap)
    wk = np.zeros((KS, C, C), f)
    for o in range(C):
        g = o // 32
        wk[:, 32 * g:32 * (g + 1), o] = (w_key[o].T * s_k[o])
    d["wkey"] = np.ascontiguousarray(
        wk.transpose(1, 0, 2)).reshape(C, KS * C)        # (i, tap*o)
    d["tk"] = t_k.reshape(C, 1)

    s_e = (inp["bne_g"] / np.sqrt(inp["bne_v"] + EPS)).astype(f)
    t_e = (inp["bne_b"] - inp["bne_m"] * s_e).astype(f)
    we1 = inp["w_e1"] * s_e[:, None]                      # (64, 256)
    d["we1a"] = np.ascontiguousarray(we1[:, :C].T)
    d["we1b"] = np.ascontiguousarray(we1[:, C:].T)
    d["te2"] = np.concatenate([t_e, t_e]).reshape(C, 1)
    we2t = np.ascontiguousarray(inp["w_e2"].T)           # (64, 96)
    d["we2"] = np.concatenate([we2t, we2t], axis=0)      # stacked twice

    s_1 = (inp["bn1_g"] / np.sqrt(inp["bn1_v"] + EPS)).astype(f)
    t_1 = (inp["bn1_b"] - inp["bn1_m"] * s_1).astype(f)
    d["wv"] = np.ascontiguousarray((inp["w_1x1"] * s_1[:, None]).T)
    # tvc built per-core in _shard_inputs (pad slot differs by parity)
    d["_t1"] = t_1

    gn_g, gn_b, b_e2 = inp["gn_g"], inp["gn_b"], inp["b_e2"]
    rows = np.arange(KC)
    cols = np.arange(C)
    bkg = np.zeros((KS, KC, C), f)
    for k in range(KS):
        bkg[k] = (rows[:, None] == (3 * (cols[None, :] // 4) + k)) * \
            gn_g[rows][:, None]
    d["bkg"] = np.ascontiguousarray(
        bkg.transpose(1, 0, 2)).reshape(KC, KS * C)      # (r, tap*c)
    d["gavg"] = ((rows[:, None] // 3 == np.arange(G)[None, :]) /
                 np.float32(3.0)).astype(f)
    d["c1"] = (np.arange(G)[:, None] == (cols[None, :] // 4)).astype(f)
    d["c96"] = (np.arange(G)[:, None] == (rows[None, :] // 3)).astype(f)
    d["be2"] = b_e2.astype(f).reshape(KC, 1)
    ge = 3 * (cols // 4)
    ecols = np.zeros((C, KS, 3), f)
    for k in range(KS):
        ecols[:, k, 0] = b_e2[ge + k]
        ecols[:, k, 1] = gn_g[ge + k]
        ecols[:, k, 2] = gn_b[ge + k]
    d["ecols"] = ecols.reshape(C, KS * 3)

    s_2 = (inp["bn2_g"] / np.sqrt(inp["bn2_v"] + EPS)).astype(f)
    d["s2"] = s_2.reshape(C, 1)
    d["t2"] = (inp["bn2_b"] - inp["bn2_m"] * s_2).astype(f).reshape(C, 1)

    s_se = (inp["bnse_g"] / np.sqrt(inp["bnse_v"] + EPS)).astype(f)
    # gap arrives as the raw sum over the full clip; the 1/(N*H*W) mean
    # is applied as the activation scale on the SE input
    d["wse1"] = np.ascontiguousarray((inp["w_se1"] * s_se[:, None]).T)
    d["b1"] = (s_se * inp["b_se1"] +
               (inp["bnse_b"] - inp["bnse_m"] * s_se)).astype(f).reshape(C, 1)
    w2 = inp["w_se2"]
    d["wsed"] = np.ascontiguousarray((w2[0::2, :] - w2[1::2, :]).T)
    d["bd"] = (inp["b_se2"][0::2] - inp["b_se2"][1::2]).astype(f).reshape(C, 1)
    return d


def _pack_blobs(d):
    cb16 = np.zeros((C, XT - B16), np.float16)
    for nm, (r, c0, cn) in L16.items():
        v = np.asarray(d[nm], np.float32)
        assert v.shape == (r, cn), (nm, v.shape, (r, cn))
        cb16[0:r, c0 - B16:c0 - B16 + cn] = v.astype(np.float16)
    for nm, (r, c0, cn) in L32.items():
        if nm in ("tvc", "tmask"):
            continue
        v = np.asarray(d[nm], np.float32)
        assert v.shape == (r, cn), (nm, v.shape, (r, cn))
        cb16[0:r, NC16 + c0:NC16 + c0 + cn] = v.astype(np.float16)
    # slot-uniform v bias; per-core tap masks filled in _shard_inputs
    tvc_c = L32["tvc"][1]
    cb16[:, NC16 + tvc_c:NC16 + tvc_c + S] = \
        d["_t1"].astype(np.float16)[:, None]
    cb16[:, NC16 + L32["tmask"][1]:NC16 + L32["tmask"][1] + KS * L] = 1.0
    return cb16


def _shard_inputs(inputs):
    d = _host_constants(inputs)
    cb16 = _pack_blobs(d)
    x = np.asarray(inputs["x"], np.float32)
    x5 = x.reshape(B, NB, C, PXF).astype(np.float16)

    msk_c = L32["tmask"][1]
    in_maps = []
    for core in range(NCORES):
        clip, half = core // 2, core % 2
        xall = np.zeros((C, XT), np.float16)
        xall[:, B16:] = cb16
        xb = xall[:, 0:B16].reshape(C, NT, S, PT)
        for j in range(S):
            gf = half * 4 + j - 1
            if 0 <= gf < NB:
                fr = x5[clip, gf]                        # [C, PXF]
                xb[:, :, j, :] = fr.reshape(C, NT, PT)
        # kill the tap that reads the clip-edge pad slot: (k,n)=(0,0) on
        # even halves (slot 0 = frame -1), (2,3) on odd (slot 5 = frame 8)
        bad = 0 * L + 0 if half == 0 else 2 * L + 3
        xall[:, B16 + NC16 + msk_c + bad] = 0.0
        in_maps.append({"xall": np.ascontiguousarray(xall)})
    return in_maps


def kernel(**inputs):
    if "nc" not in _CACHE:
        _CACHE["nc"] = _build_program()
    nc = _CACHE["nc"]

    in_maps = _shard_inputs(inputs)
    # the axon-tunneled runtime occasionally throws a transient error on a
    # session's first dispatches; one retry after a short settle fixes it
    import time
    for attempt in range(3):
        try:
            res = run_bass_kernel_spmd(nc, in_maps, list(range(NCORES)))
            break
        except Exception:
            if attempt == 2:
                raise
            time.sleep(3.0)

    out = np.empty((B, NB, C, H, W), np.float32)
    for core in range(NCORES):
        clip, half = core // 2, core % 2
        o = res.results[core]["out"].reshape(C, L, PXF)
        for i in range(L):
            out[clip, half * 4 + i] = o[:, i].astype(np.float32).reshape(
                C, H, W)
    return out.reshape(B * NB, C, H, W)


if __name__ == "__main__":
    sys.path.insert(0, "/root/problem")
    import reference
    inp = {k: np.asarray(v) for k, v in reference.setup_inputs().items()}
    got = kernel(**inp)
    exp = np.asarray(reference.reference(**inp))
    err = np.abs(got - exp).max() / np.abs(exp).max()
    print("abs-max relative error:", err)
